# revision 4
# baseline (speedup 1.0000x reference)
"""Trainium2 Bass kernel for nn_Attention_9594956939856.

Single-head spatial self-attention over 64x64 feature maps:
    q = Wq@x, k = Wk@x, v = Wv@x  (1x1 convs)
    out = gamma * softmax(q^T k) @ v + x

Sharding: data-parallel over batch — 8 samples onto 8 NeuronCores, each core
computes one full sample (C=256, N=4096 tokens, dk=32). No collectives.

Per-core layout strategy (all matmuls on TensorE as out = lhsT.T @ rhs):
  - scores are computed directly TRANSPOSED: s'[j,i] = sum_d k[d,j] q[d,i]
    with k j-tiles as the stationary operand, so no transposes of the huge
    attention matrix are ever needed.
  - softmax denominator: ones(128,128) as stationary sums exp(s') over
    partitions (j), accumulated across j-tiles in PSUM; the M=128 ones matrix
    broadcasts the sum to all output partitions for free.
  - v is produced in transposed layout vT[n,e] directly by the projection
    (lhsT = x chunks, rhs = Wv^T), which is exactly the layout the
    attention-weighted sum needs as the stationary operand.
  - exp on ScalarE (bf16 out), everything accumulates fp32 in PSUM.
    Scores are in [-5, 5] for this distribution so the max-subtraction-free
    softmax is numerically safe.
"""

import numpy as np

import concourse.bass as bass
import concourse.mybir as mybir
from concourse.tile import TileContext
from concourse.bass_utils import run_bass_kernel_spmd

B, C, H, W = 8, 256, 64, 64
N = H * W          # 4096 tokens
DK = C // 8        # 32
P = 128
F32 = mybir.dt.float32
BF16 = mybir.dt.bfloat16
AF = mybir.ActivationFunctionType
ALU = mybir.AluOpType

NJT = N // P       # 32 j-tiles
ICH = 1024         # i-chunk width for the scores'/exp stage
NICH = N // ICH    # 4
HCH = 512          # accumulation sub-chunk (one PSUM bank)


# ---------------------------------------------------------------------------
# Workaround: the walrus build in this container allows only ONE sync wait
# per instruction ("Too many sync wait commands"), but Tile's wait
# assignment attaches up to 2 (and the tail drain even more). Split: hoist
# all-but-one wait of any over-subscribed instruction onto dedicated
# same-engine nofuse nops inserted immediately before it; same for the
# kernel-tail drain.
_PATCHED = False


def _apply_tile_patch():
    global _PATCHED
    if _PATCHED:
        return
    from concourse.tile import TileContext as TC
    from concourse.vector_clock import ScopedClock, VectorClock

    def _drain_and_barrier_split(self, tick_clock, wait_clock):
        gc = tick_clock.global_clock
        n = len(gc)
        for i in range(n):
            if gc[i] > 0:
                vec = [0] * n
                vec[i] = gc[i]
                ins = self.nc.sync.nop(nofuse=True, hint="tail_drain_wait")
                wait_clock.add_sem_waits(
                    ins.ins, ScopedClock({None: VectorClock(vec)})
                )
        self.nc.sync.drain()
        self.nc.all_engine_barrier()
        assert self.sems is not None
        popped = self.nc._tile_sem_poison_stack.pop()
        assert popped is self._sem_poison
        self.nc.clear_and_free_semaphores(list(self.sems.allocated().values()))
        self.nc.all_engine_barrier()

    TC._drain_and_barrier = _drain_and_barrier_split

    orig_lower = TC._lower_ordered_insts
    counter = [0]

    def _lower_split_waits(self, ordered):
        for bb_name, insts in ordered.items():
            new = []
            changed = False
            for inst in insts:
                si = inst.sync_info
                if si is not None and len(si.on_wait) > 1:
                    changed = True
                    waits = list(si.on_wait)
                    for w in waits[:-1]:
                        counter[0] += 1
                        new.append(
                            mybir.InstNoOp(
                                name=f"splitw-{counter[0]}",
                                sync_info=mybir.SyncInfo(
                                    on_wait=[w], on_update=[]
                                ),
                                bass_nofuse=True,
                                engine=inst.engine,
                            )
                        )
                    inst.sync_info = mybir.SyncInfo(
                        on_wait=[waits[-1]], on_update=list(si.on_update)
                    )
                new.append(inst)
            if changed:
                insts[:] = new
        return orig_lower(self, ordered)

    TC._lower_ordered_insts = _lower_split_waits
    _PATCHED = True


def build_bass(n_reps: int = 1) -> bass.Bass:
    """Build the kernel. n_reps > 1 repeats the whole computation in one NEFF
    (idempotent — same inputs/outputs) for slope-based device timing."""
    _apply_tile_patch()
    nc = bass.Bass()

    x_e = nc.declare_dram_parameter("x", [C, N], F32, isOutput=False)
    wqt_e = nc.declare_dram_parameter("wqt", [C, P], F32, isOutput=False)
    wkt_e = nc.declare_dram_parameter("wkt", [C, P], F32, isOutput=False)
    wvt_e = nc.declare_dram_parameter("wvt", [C, C], F32, isOutput=False)
    bq_e = nc.declare_dram_parameter("bq_r", [P, 1], F32, isOutput=False)
    bk_e = nc.declare_dram_parameter("bk_r", [P, 1], F32, isOutput=False)
    bv_e = nc.declare_dram_parameter("bv_b", [P, C], F32, isOutput=False)
    gam_e = nc.declare_dram_parameter("gam_b", [P, 1], F32, isOutput=False)
    y_e = nc.declare_dram_parameter("y", [C, N], F32, isOutput=True)

    with (
        TileContext(nc) as tc,
        tc.tile_pool(name="consts", bufs=1) as consts,
        tc.tile_pool(name="big", bufs=1) as big,
        tc.tile_pool(name="epool", bufs=36) as epool,
        tc.tile_pool(name="fin", bufs=4) as fin,
        tc.tile_pool(name="ps_s", bufs=2, space="PSUM") as ps_s_pool,
        tc.tile_pool(name="ps_acc", bufs=3, space="PSUM") as ps_acc_pool,
    ):
      for _rep in range(n_reps):
        # ---- constants / weights -----------------------------------------
        wqt_a = consts.tile([P, P], F32)
        wqt_b = consts.tile([P, P], F32)
        wkt_a = consts.tile([P, P], F32)
        wkt_b = consts.tile([P, P], F32)
        wvt_a = consts.tile([P, C], F32)
        wvt_b = consts.tile([P, C], F32)
        bq_t = consts.tile([P, 1], F32)
        bk_t = consts.tile([P, 1], F32)
        bv_t = consts.tile([P, C], F32)
        gam_t = consts.tile([P, 1], F32)
        ones = consts.tile([P, P], BF16)

        nc.sync.dma_start(out=wqt_a[:], in_=wqt_e[0:P, :])
        nc.sync.dma_start(out=wqt_b[:], in_=wqt_e[P : 2 * P, :])
        nc.sync.dma_start(out=wkt_a[:], in_=wkt_e[0:P, :])
        nc.sync.dma_start(out=wkt_b[:], in_=wkt_e[P : 2 * P, :])
        nc.sync.dma_start(out=wvt_a[:], in_=wvt_e[0:P, :])
        nc.sync.dma_start(out=wvt_b[:], in_=wvt_e[P : 2 * P, :])
        nc.sync.dma_start(out=bq_t[:], in_=bq_e[:])
        nc.sync.dma_start(out=bk_t[:], in_=bk_e[:])
        nc.sync.dma_start(out=bv_t[:], in_=bv_e[:])
        nc.sync.dma_start(out=gam_t[:], in_=gam_e[:])
        nc.vector.memset(ones[:], 1.0)

        xf0 = big.tile([P, N], F32)
        xf1 = big.tile([P, N], F32)
        nc.sync.dma_start(out=xf0[:], in_=x_e[0:P, :])
        nc.sync.dma_start(out=xf1[:], in_=x_e[P : 2 * P, :])

        # ---- projections --------------------------------------------------
        # q_rep/k_rep: (dk, N) replicated 4x along partitions (wqt/wkt are
        # host-side replicated W^T so the matmul output lands replicated).
        q_rep = big.tile([P, N], BF16)
        k_rep = big.tile([P, N], BF16)
        for nch in range(N // HCH):
            sl = slice(nch * HCH, (nch + 1) * HCH)
            pq = ps_acc_pool.tile([P, HCH], F32, tag="acc")
            nc.tensor.matmul(pq[:], wqt_a[:], xf0[:, sl], start=True, stop=False)
            nc.tensor.matmul(pq[:], wqt_b[:], xf1[:, sl], start=False, stop=True)
            nc.vector.tensor_scalar_add(q_rep[:, sl], pq[:], bq_t[:])
            pk = ps_acc_pool.tile([P, HCH], F32, tag="acc")
            nc.tensor.matmul(pk[:], wkt_a[:], xf0[:, sl], start=True, stop=False)
            nc.tensor.matmul(pk[:], wkt_b[:], xf1[:, sl], start=False, stop=True)
            nc.vector.tensor_scalar_add(k_rep[:, sl], pk[:], bk_t[:])

        # vT: per j-tile (n on partitions, channel on free) = x^T @ Wv^T + bv
        vt = big.tile([P, NJT * C], BF16)
        for jt in range(NJT):
            nsl = slice(jt * P, (jt + 1) * P)
            pv = ps_acc_pool.tile([P, C], F32, tag="acc")
            nc.tensor.matmul(pv[:], xf0[:, nsl], wvt_a[:], start=True, stop=False)
            nc.tensor.matmul(pv[:], xf1[:, nsl], wvt_b[:], start=False, stop=True)
            nc.vector.tensor_tensor(
                vt[:, jt * C : (jt + 1) * C], pv[:], bv_t[:], op=ALU.add
            )

        # ---- attention main loop ------------------------------------------
        for ich in range(NICH):
            i0 = ich * ICH
            # stage 1: s'[j,i] per j-tile, exp -> e tiles (bf16)
            etiles = []
            for jt in range(NJT):
                ksl = k_rep[0:DK, jt * P : (jt + 1) * P]
                ps = ps_s_pool.tile([P, ICH], F32, tag="ps_s")
                for h in range(ICH // HCH):
                    qsl = q_rep[0:DK, i0 + h * HCH : i0 + (h + 1) * HCH]
                    nc.tensor.matmul(
                        ps[:, h * HCH : (h + 1) * HCH],
                        ksl,
                        qsl,
                        start=True,
                        stop=True,
                    )
                e_t = epool.tile([P, ICH], BF16, tag="e")
                nc.scalar.activation(e_t[:], ps[:], AF.Exp)
                etiles.append(e_t)

            # stage 2: accumulate out_un (2 channel halves) + denominator
            for h in range(ICH // HCH):
                hsl = slice(i0 + h * HCH, i0 + (h + 1) * HCH)
                po0 = ps_acc_pool.tile([P, HCH], F32, tag="acc")
                po1 = ps_acc_pool.tile([P, HCH], F32, tag="acc")
                pd = ps_acc_pool.tile([P, HCH], F32, tag="acc")
                for jt in range(NJT):
                    esl = etiles[jt][:, h * HCH : (h + 1) * HCH]
                    st = jt == 0
                    sp = jt == NJT - 1
                    nc.tensor.matmul(
                        po0[:], vt[:, jt * C : jt * C + P], esl, start=st, stop=sp
                    )
                    nc.tensor.matmul(
                        po1[:], vt[:, jt * C + P : (jt + 1) * C], esl,
                        start=st, stop=sp,
                    )
                    nc.tensor.matmul(pd[:], ones[:], esl, start=st, stop=sp)

                # finalize: y = gamma * out_un / den + x
                dr = fin.tile([P, HCH], F32, tag="dr")
                nc.vector.reciprocal(dr[:], pd[:])
                nc.vector.tensor_scalar_mul(dr[:], dr[:], gam_t[:])
                t0 = fin.tile([P, HCH], F32, tag="t0")
                nc.vector.tensor_tensor(t0[:], po0[:], dr[:], op=ALU.mult)
                nc.vector.tensor_tensor(t0[:], t0[:], xf0[:, hsl], op=ALU.add)
                nc.sync.dma_start(out=y_e[0:P, hsl], in_=t0[:])
                t1 = fin.tile([P, HCH], F32, tag="t1")
                nc.vector.tensor_tensor(t1[:], po1[:], dr[:], op=ALU.mult)
                nc.vector.tensor_tensor(t1[:], t1[:], xf1[:, hsl], op=ALU.add)
                nc.sync.dma_start(out=y_e[P : 2 * P, hsl], in_=t1[:])

    return nc


_NC_CACHE = None


def _get_nc() -> bass.Bass:
    global _NC_CACHE
    if _NC_CACHE is None:
        _NC_CACHE = build_bass()
    return _NC_CACHE


def prep_core_inputs(x, Wq, bq, Wk, bk, Wv, bv, gamma):
    x = np.asarray(x, np.float32).reshape(B, C, N)
    wqt = np.ascontiguousarray(np.tile(np.asarray(Wq, np.float32).T, (1, 4)))
    wkt = np.ascontiguousarray(np.tile(np.asarray(Wk, np.float32).T, (1, 4)))
    wvt = np.ascontiguousarray(np.asarray(Wv, np.float32).T)
    bq_r = np.ascontiguousarray(np.tile(np.asarray(bq, np.float32), 4)).reshape(P, 1)
    bk_r = np.ascontiguousarray(np.tile(np.asarray(bk, np.float32), 4)).reshape(P, 1)
    bv_b = np.ascontiguousarray(np.broadcast_to(np.asarray(bv, np.float32), (P, C)))
    gam_b = np.full((P, 1), float(np.asarray(gamma).reshape(-1)[0]), np.float32)
    shared = {
        "wqt": wqt, "wkt": wkt, "wvt": wvt,
        "bq_r": bq_r, "bk_r": bk_r, "bv_b": bv_b, "gam_b": gam_b,
    }
    return [{"x": np.ascontiguousarray(x[b]), **shared} for b in range(B)]


def kernel(**inputs) -> np.ndarray:
    nc = _get_nc()
    in_maps = prep_core_inputs(**inputs)
    res = run_bass_kernel_spmd(nc, in_maps, list(range(B)))
    y = np.stack([res.results[i]["y"] for i in range(B)])
    return np.ascontiguousarray(y.reshape(B, C, H, W).astype(np.float32))


# revision 5
# speedup vs baseline: 10.2425x; 10.2425x over previous
"""Trainium2 Bass kernel for nn_Attention_9594956939856.

Single-head spatial self-attention over 64x64 feature maps:
    q = Wq@x, k = Wk@x, v = Wv@x  (1x1 convs over channels)
    out = gamma * softmax(q^T k) @ v + x

Sharding: data-parallel over batch — 8 samples onto 8 NeuronCores, each core
computes one full sample (C=256, N=4096 tokens, dk=32). No collectives.

Per-core layout strategy (matmuls on TensorE compute out = lhsT.T @ rhs):
  - scores are computed directly TRANSPOSED: s'[j,i] = sum_d k[d,j] q[d,i]
    with k j-tiles stationary, so the huge attention matrix never needs a
    transpose. q/k are replicated 4x along partitions (via host-replicated
    W^T) so the K=32 contraction can later use 4x row-tiled matmuls.
  - softmax denominator: ones(128,128) stationary sums exp(s') over
    partitions (j), accumulated across j-tiles in PSUM; M=128 broadcasts the
    sum to every output partition for free.
  - v is produced directly in transposed layout vT[n,e] by the projection
    (lhsT = x chunks, rhs = Wv^T) — exactly the stationary layout the
    attention-weighted sum needs.
  - exp on ScalarE in (128,1024) chunks (bf16 out), fp32 PSUM accumulation.
    Scores are in [-5,5] for this input distribution, so softmax without
    max-subtraction is numerically safe.
"""

import numpy as np

import concourse.bass as bass
import concourse.mybir as mybir
from concourse.tile import TileContext
from concourse.bass_utils import run_bass_kernel_spmd

B, C, H, W = 8, 256, 64, 64
N = H * W          # 4096 tokens
DK = C // 8        # 32
P = 128
F32 = mybir.dt.float32
BF16 = mybir.dt.bfloat16
AF = mybir.ActivationFunctionType
ALU = mybir.AluOpType

NJT = N // P       # 32 j-tiles
ICH = 1024         # i-chunk width for the scores'/exp stage
NICH = N // ICH    # 4
HCH = 512          # accumulation sub-chunk (one PSUM bank)


# ---------------------------------------------------------------------------
# Workaround: the walrus build in this container allows only ONE sync wait
# per instruction ("Too many sync wait commands"), but Tile's wait
# assignment attaches up to 2 (and the tail drain more). Hoist all-but-one
# wait of any over-subscribed instruction onto dedicated same-engine nofuse
# nops inserted immediately before it in the ordered stream.
_PATCHED = False


def _apply_tile_patch():
    global _PATCHED
    if _PATCHED:
        return
    from concourse.tile import TileContext as TC
    from concourse.vector_clock import ScopedClock, VectorClock

    def _drain_and_barrier_split(self, tick_clock, wait_clock):
        gc = tick_clock.global_clock
        n = len(gc)
        for i in range(n):
            if gc[i] > 0:
                vec = [0] * n
                vec[i] = gc[i]
                ins = self.nc.sync.nop(nofuse=True, hint="tail_drain_wait")
                wait_clock.add_sem_waits(
                    ins.ins, ScopedClock({None: VectorClock(vec)})
                )
        self.nc.sync.drain()
        self.nc.all_engine_barrier()
        assert self.sems is not None
        popped = self.nc._tile_sem_poison_stack.pop()
        assert popped is self._sem_poison
        self.nc.clear_and_free_semaphores(list(self.sems.allocated().values()))
        self.nc.all_engine_barrier()

    TC._drain_and_barrier = _drain_and_barrier_split

    orig_lower = TC._lower_ordered_insts
    counter = [0]

    def _lower_split_waits(self, ordered):
        for bb_name, insts in ordered.items():
            new = []
            changed = False
            for inst in insts:
                si = inst.sync_info
                if si is not None and len(si.on_wait) > 1:
                    changed = True
                    waits = list(si.on_wait)
                    for w in waits[:-1]:
                        counter[0] += 1
                        new.append(
                            mybir.InstNoOp(
                                name=f"splitw-{counter[0]}",
                                sync_info=mybir.SyncInfo(
                                    on_wait=[w], on_update=[]
                                ),
                                bass_nofuse=True,
                                engine=inst.engine,
                            )
                        )
                    inst.sync_info = mybir.SyncInfo(
                        on_wait=[waits[-1]], on_update=list(si.on_update)
                    )
                new.append(inst)
            if changed:
                insts[:] = new
        return orig_lower(self, ordered)

    TC._lower_ordered_insts = _lower_split_waits
    _PATCHED = True


def _emit_body(nc, tc, pools, ext):
    """Emit one full attention computation (one sample)."""
    consts, big, epool, fin, ps_s_pool, ps_acc_pool = pools
    x_e, wqt_e, wkt_e, wvt_e, bq_e, bk_e, bv_e, gam_e, y_e = ext

    # ---- constants / weights ---------------------------------------------
    wqt_a = consts.tile([P, P], F32, tag="wqt_a")
    wqt_b = consts.tile([P, P], F32, tag="wqt_b")
    wkt_a = consts.tile([P, P], F32, tag="wkt_a")
    wkt_b = consts.tile([P, P], F32, tag="wkt_b")
    wvt_a = consts.tile([P, C], F32, tag="wvt_a")
    wvt_b = consts.tile([P, C], F32, tag="wvt_b")
    bq_t = consts.tile([P, 1], F32, tag="bq_t")
    bk_t = consts.tile([P, 1], F32, tag="bk_t")
    bv_t = consts.tile([P, C], F32, tag="bv_t")
    gam_t = consts.tile([P, 1], F32, tag="gam_t")
    ones = consts.tile([P, P], BF16, tag="ones")

    nc.sync.dma_start(out=wqt_a[:], in_=wqt_e[0:P, :])
    nc.sync.dma_start(out=wqt_b[:], in_=wqt_e[P : 2 * P, :])
    nc.sync.dma_start(out=wkt_a[:], in_=wkt_e[0:P, :])
    nc.sync.dma_start(out=wkt_b[:], in_=wkt_e[P : 2 * P, :])
    nc.sync.dma_start(out=wvt_a[:], in_=wvt_e[0:P, :])
    nc.sync.dma_start(out=wvt_b[:], in_=wvt_e[P : 2 * P, :])
    nc.sync.dma_start(out=bq_t[:], in_=bq_e[:])
    nc.sync.dma_start(out=bk_t[:], in_=bk_e[:])
    nc.sync.dma_start(out=bv_t[:], in_=bv_e[:])
    nc.sync.dma_start(out=gam_t[:], in_=gam_e[:])
    nc.vector.memset(ones[:], 1.0)

    xf0 = big.tile([P, N], F32, tag="xf0")
    xf1 = big.tile([P, N], F32, tag="xf1")
    nc.sync.dma_start(out=xf0[:], in_=x_e[0:P, :])
    nc.sync.dma_start(out=xf1[:], in_=x_e[P : 2 * P, :])

    # ---- projections ------------------------------------------------------
    q_rep = big.tile([P, N], BF16, tag="q_rep")
    k_rep = big.tile([P, N], BF16, tag="k_rep")
    for nch in range(N // HCH):
        sl = slice(nch * HCH, (nch + 1) * HCH)
        pq = ps_acc_pool.tile([P, HCH], F32, tag="acc")
        nc.tensor.matmul(pq[:], wqt_a[:], xf0[:, sl], start=True, stop=False)
        nc.tensor.matmul(pq[:], wqt_b[:], xf1[:, sl], start=False, stop=True)
        nc.vector.tensor_scalar_add(q_rep[:, sl], pq[:], bq_t[:])
        pk = ps_acc_pool.tile([P, HCH], F32, tag="acc")
        nc.tensor.matmul(pk[:], wkt_a[:], xf0[:, sl], start=True, stop=False)
        nc.tensor.matmul(pk[:], wkt_b[:], xf1[:, sl], start=False, stop=True)
        nc.vector.tensor_scalar_add(k_rep[:, sl], pk[:], bk_t[:])

    # vT per j-tile: (n on partitions, channel on free) = x^T @ Wv^T + bv
    vt = big.tile([P, NJT * C], BF16, tag="vt")
    for jt in range(NJT):
        nsl = slice(jt * P, (jt + 1) * P)
        pv = ps_acc_pool.tile([P, C], F32, tag="acc")
        nc.tensor.matmul(pv[:], xf0[:, nsl], wvt_a[:], start=True, stop=False)
        nc.tensor.matmul(pv[:], xf1[:, nsl], wvt_b[:], start=False, stop=True)
        nc.vector.tensor_tensor(
            vt[:, jt * C : (jt + 1) * C], pv[:], bv_t[:], op=ALU.add
        )

    # ---- attention main loop ---------------------------------------------
    for ich in range(NICH):
        i0 = ich * ICH
        # stage 1: s'[j,i] per j-tile, exp -> e tiles (bf16)
        etiles = []
        for jt in range(NJT):
            ksl = k_rep[0:DK, jt * P : (jt + 1) * P]
            ps = ps_s_pool.tile([P, ICH], F32, tag="ps_s")
            for h in range(ICH // HCH):
                qsl = q_rep[0:DK, i0 + h * HCH : i0 + (h + 1) * HCH]
                nc.tensor.matmul(
                    ps[:, h * HCH : (h + 1) * HCH], ksl, qsl,
                    start=True, stop=True,
                )
            e_t = epool.tile([P, ICH], BF16, tag="e")
            nc.scalar.activation(e_t[:], ps[:], AF.Exp)
            etiles.append(e_t)

        # stage 2: accumulate out_un (2 channel halves) + denominator
        for h in range(ICH // HCH):
            hsl = slice(i0 + h * HCH, i0 + (h + 1) * HCH)
            po0 = ps_acc_pool.tile([P, HCH], F32, tag="acc")
            po1 = ps_acc_pool.tile([P, HCH], F32, tag="acc")
            pd = ps_acc_pool.tile([P, HCH], F32, tag="acc")
            for jt in range(NJT):
                esl = etiles[jt][:, h * HCH : (h + 1) * HCH]
                st = jt == 0
                sp = jt == NJT - 1
                nc.tensor.matmul(
                    po0[:], vt[:, jt * C : jt * C + P], esl, start=st, stop=sp
                )
                nc.tensor.matmul(
                    po1[:], vt[:, jt * C + P : (jt + 1) * C], esl,
                    start=st, stop=sp,
                )
                nc.tensor.matmul(pd[:], ones[:], esl, start=st, stop=sp)

            # finalize: y = gamma * out_un / den + x
            dr = fin.tile([P, HCH], F32, tag="dr")
            nc.vector.reciprocal(dr[:], pd[:])
            nc.vector.tensor_scalar_mul(dr[:], dr[:], gam_t[:])
            t0 = fin.tile([P, HCH], F32, tag="t0")
            nc.vector.tensor_tensor(t0[:], po0[:], dr[:], op=ALU.mult)
            nc.vector.tensor_tensor(t0[:], t0[:], xf0[:, hsl], op=ALU.add)
            nc.sync.dma_start(out=y_e[0:P, hsl], in_=t0[:])
            t1 = fin.tile([P, HCH], F32, tag="t1")
            nc.vector.tensor_tensor(t1[:], po1[:], dr[:], op=ALU.mult)
            nc.vector.tensor_tensor(t1[:], t1[:], xf1[:, hsl], op=ALU.add)
            nc.sync.dma_start(out=y_e[P : 2 * P, hsl], in_=t1[:])


def build_bass(loop_n: int | None = None) -> bass.Bass:
    """Build the kernel. loop_n wraps the body in a device-side For_i loop
    (with a tiny 'tick' sentinel output) for slope-based benchmarking."""
    _apply_tile_patch()
    nc = bass.Bass()

    x_e = nc.declare_dram_parameter("x", [C, N], F32, isOutput=False)
    wqt_e = nc.declare_dram_parameter("wqt", [C, P], F32, isOutput=False)
    wkt_e = nc.declare_dram_parameter("wkt", [C, P], F32, isOutput=False)
    wvt_e = nc.declare_dram_parameter("wvt", [C, C], F32, isOutput=False)
    bq_e = nc.declare_dram_parameter("bq_r", [P, 1], F32, isOutput=False)
    bk_e = nc.declare_dram_parameter("bk_r", [P, 1], F32, isOutput=False)
    bv_e = nc.declare_dram_parameter("bv_b", [P, C], F32, isOutput=False)
    gam_e = nc.declare_dram_parameter("gam_b", [P, 1], F32, isOutput=False)
    y_e = nc.declare_dram_parameter("y", [C, N], F32, isOutput=True)
    tick_e = None
    if loop_n is not None:
        tick_e = nc.declare_dram_parameter("tick", [1, 8], F32, isOutput=True)

    ext = (x_e, wqt_e, wkt_e, wvt_e, bq_e, bk_e, bv_e, gam_e, y_e)

    with (
        TileContext(nc) as tc,
        tc.tile_pool(name="consts", bufs=1) as consts,
        tc.tile_pool(name="big", bufs=1) as big,
        tc.tile_pool(name="epool", bufs=36) as epool,
        tc.tile_pool(name="fin", bufs=4) as fin,
        tc.tile_pool(name="ps_s", bufs=2, space="PSUM") as ps_s_pool,
        tc.tile_pool(name="ps_acc", bufs=3, space="PSUM") as ps_acc_pool,
    ):
        pools = (consts, big, epool, fin, ps_s_pool, ps_acc_pool)
        if loop_n is None:
            _emit_body(nc, tc, pools, ext)
        else:
            with tc.For_i(0, loop_n, 1):
                _emit_body(nc, tc, pools, ext)
            t = fin.tile([1, 8], F32, tag="tick")
            nc.vector.memset(t[:], 1.0)
            nc.sync.dma_start(out=tick_e[:], in_=t[:])

    return nc


_NC_CACHE = None


def _get_nc() -> bass.Bass:
    global _NC_CACHE
    if _NC_CACHE is None:
        _NC_CACHE = build_bass()
    return _NC_CACHE


def prep_core_inputs(x, Wq, bq, Wk, bk, Wv, bv, gamma):
    x = np.asarray(x, np.float32).reshape(B, C, N)
    wqt = np.ascontiguousarray(np.tile(np.asarray(Wq, np.float32).T, (1, 4)))
    wkt = np.ascontiguousarray(np.tile(np.asarray(Wk, np.float32).T, (1, 4)))
    wvt = np.ascontiguousarray(np.asarray(Wv, np.float32).T)
    bq_r = np.ascontiguousarray(np.tile(np.asarray(bq, np.float32), 4)).reshape(P, 1)
    bk_r = np.ascontiguousarray(np.tile(np.asarray(bk, np.float32), 4)).reshape(P, 1)
    bv_b = np.ascontiguousarray(np.broadcast_to(np.asarray(bv, np.float32), (P, C)))
    gam_b = np.full((P, 1), float(np.asarray(gamma).reshape(-1)[0]), np.float32)
    shared = {
        "wqt": wqt, "wkt": wkt, "wvt": wvt,
        "bq_r": bq_r, "bk_r": bk_r, "bv_b": bv_b, "gam_b": gam_b,
    }
    return [{"x": np.ascontiguousarray(x[b]), **shared} for b in range(B)]


def kernel(**inputs) -> np.ndarray:
    nc = _get_nc()
    in_maps = prep_core_inputs(**inputs)
    res = run_bass_kernel_spmd(nc, in_maps, list(range(B)))
    y = np.stack([res.results[i]["y"] for i in range(B)])
    return np.ascontiguousarray(y.reshape(B, C, H, W).astype(np.float32))


# revision 15
# speedup vs baseline: 13.6227x; 1.3300x over previous
"""Trainium2 Bass kernel for nn_Attention_9594956939856.

Single-head spatial self-attention over 64x64 feature maps:
    q = Wq@x, k = Wk@x, v = Wv@x  (1x1 convs over channels)
    out = gamma * softmax(q^T k) @ v + x

Sharding: data-parallel over batch — 8 samples onto 8 NeuronCores, each core
computes one full sample (C=256, N=4096 tokens, dk=32). No collectives.

Per-core layout strategy (matmuls on TensorE compute out = lhsT.T @ rhs):
  - scores are computed directly TRANSPOSED: s'[j,i] = sum_d k[d,j] q[d,i]
    with k j-tiles stationary, so the huge attention matrix never needs a
    transpose. q/k are replicated 4x along partitions (via host-replicated
    W^T) so the K=32 contraction can later use 4x row-tiled matmuls.
  - softmax denominator: ones(128,128) stationary sums exp(s') over
    partitions (j), accumulated across j-tiles in PSUM; M=128 broadcasts the
    sum to every output partition for free.
  - v is produced directly in transposed layout vT[n,e] by the projection
    (lhsT = x chunks, rhs = Wv^T) — exactly the stationary layout the
    attention-weighted sum needs.
  - exp on ScalarE in (128,1024) chunks (bf16 out), fp32 PSUM accumulation.
    Scores are in [-5,5] for this input distribution, so softmax without
    max-subtraction is numerically safe.
"""

import ml_dtypes
import numpy as np

import concourse.bass as bass
import concourse.mybir as mybir
from concourse.tile import TileContext
from concourse.bass_utils import run_bass_kernel_spmd

B, C, H, W = 8, 256, 64, 64
N = H * W          # 4096 tokens
DK = C // 8        # 32
P = 128
F32 = mybir.dt.float32
F32R = mybir.dt.float32r  # fp32 storage, single-pass (4x faster) PE streaming
BF16 = mybir.dt.bfloat16
AF = mybir.ActivationFunctionType
ALU = mybir.AluOpType

NJT = N // P       # 32 j-tiles
ICH = 1024         # i-chunk width for the scores'/exp stage
NICH = N // ICH    # 4
HCH = 512          # accumulation sub-chunk (one PSUM bank)


# ---------------------------------------------------------------------------
# Workaround: the walrus build in this container allows only ONE sync wait
# per instruction ("Too many sync wait commands"), but Tile's wait
# assignment attaches up to 2 (and the tail drain more). Hoist all-but-one
# wait of any over-subscribed instruction onto dedicated same-engine nofuse
# nops inserted immediately before it in the ordered stream.
_PATCHED = False


def _apply_tile_patch():
    global _PATCHED
    if _PATCHED:
        return
    from concourse.tile import TileContext as TC
    from concourse.vector_clock import ScopedClock, VectorClock

    def _drain_and_barrier_split(self, tick_clock, wait_clock):
        gc = tick_clock.global_clock
        n = len(gc)
        for i in range(n):
            if gc[i] > 0:
                vec = [0] * n
                vec[i] = gc[i]
                ins = self.nc.sync.nop(nofuse=True, hint="tail_drain_wait")
                wait_clock.add_sem_waits(
                    ins.ins, ScopedClock({None: VectorClock(vec)})
                )
        self.nc.sync.drain()
        self.nc.all_engine_barrier()
        assert self.sems is not None
        popped = self.nc._tile_sem_poison_stack.pop()
        assert popped is self._sem_poison
        self.nc.clear_and_free_semaphores(list(self.sems.allocated().values()))
        self.nc.all_engine_barrier()

    TC._drain_and_barrier = _drain_and_barrier_split

    orig_lower = TC._lower_ordered_insts
    counter = [0]

    def _lower_split_waits(self, ordered):
        for bb_name, insts in ordered.items():
            new = []
            changed = False
            for inst in insts:
                si = inst.sync_info
                if si is not None and len(si.on_wait) > 1:
                    changed = True
                    waits = list(si.on_wait)
                    for w in waits[:-1]:
                        counter[0] += 1
                        new.append(
                            mybir.InstNoOp(
                                name=f"splitw-{counter[0]}",
                                sync_info=mybir.SyncInfo(
                                    on_wait=[w], on_update=[]
                                ),
                                bass_nofuse=True,
                                engine=inst.engine,
                            )
                        )
                    inst.sync_info = mybir.SyncInfo(
                        on_wait=[waits[-1]], on_update=list(si.on_update)
                    )
                new.append(inst)
            if changed:
                insts[:] = new
        return orig_lower(self, ordered)

    TC._lower_ordered_insts = _lower_split_waits
    _PATCHED = True


def _emit_body(nc, tc, pools, ext):
    """Emit one full attention computation (one sample)."""
    consts, big, epool, fin, ps_s_pool, ps_acc_pool = pools
    x_e, wqt_e, wkt_e, wvt_e, bq_e, bk_e, bv_e, gam_e, y_e = ext

    # ---- constants / weights ---------------------------------------------
    wqt_a = consts.tile([P, P], BF16, tag="wqt_a")
    wqt_b = consts.tile([P, P], BF16, tag="wqt_b")
    wkt_a = consts.tile([P, P], BF16, tag="wkt_a")
    wkt_b = consts.tile([P, P], BF16, tag="wkt_b")
    wvt_a = consts.tile([P, C], BF16, tag="wvt_a")
    wvt_b = consts.tile([P, C], BF16, tag="wvt_b")
    bq_t = consts.tile([P, 1], F32, tag="bq_t")
    bk_t = consts.tile([P, 1], F32, tag="bk_t")
    bv_t = consts.tile([P, C], F32, tag="bv_t")
    gam_t = consts.tile([P, 1], F32, tag="gam_t")
    ones = consts.tile([P, P], BF16, tag="ones")
    ones_f = consts.tile([P, P], F32, tag="ones_f")

    nc.sync.dma_start(out=wqt_a[:], in_=wqt_e[0:P, :])
    nc.sync.dma_start(out=wqt_b[:], in_=wqt_e[P : 2 * P, :])
    nc.sync.dma_start(out=wkt_a[:], in_=wkt_e[0:P, :])
    nc.sync.dma_start(out=wkt_b[:], in_=wkt_e[P : 2 * P, :])
    nc.sync.dma_start(out=wvt_a[:], in_=wvt_e[0:P, :])
    nc.sync.dma_start(out=wvt_b[:], in_=wvt_e[P : 2 * P, :])
    nc.sync.dma_start(out=bq_t[:], in_=bq_e[:])
    nc.sync.dma_start(out=bk_t[:], in_=bk_e[:])
    nc.sync.dma_start(out=bv_t[:], in_=bv_e[:])
    nc.sync.dma_start(out=gam_t[:], in_=gam_e[:])
    nc.vector.memset(ones[:], 1.0)
    nc.vector.memset(ones_f[:], 1.0)

    xf0 = big.tile([P, N], F32, tag="xf0")
    xf1 = big.tile([P, N], F32, tag="xf1")
    nc.sync.dma_start(out=xf0[:], in_=x_e[0:P, :])
    nc.sync.dma_start(out=xf1[:], in_=x_e[P : 2 * P, :])

    # ---- projections (bf16 operands for full PE stream rate) -------------
    xb0 = big.tile([P, N], BF16, tag="xb0")
    xb1 = big.tile([P, N], BF16, tag="xb1")
    nc.vector.tensor_copy(xb0[:], xf0[:])
    nc.vector.tensor_copy(xb1[:], xf1[:])
    q_rep = big.tile([P, N], BF16, tag="q_rep")
    k_rep = big.tile([P, N], BF16, tag="k_rep")
    for nch in range(N // HCH):
        sl = slice(nch * HCH, (nch + 1) * HCH)
        pq = ps_acc_pool.tile([P, HCH], F32, tag="po", bufs=2)
        nc.tensor.matmul(
            pq[:], wqt_a[:], xb0[:, sl],
            start=True, stop=False,
        )
        nc.tensor.matmul(
            pq[:], wqt_b[:], xb1[:, sl],
            start=False, stop=True,
        )
        nc.vector.tensor_scalar_add(q_rep[:, sl], pq[:], bq_t[:])
        pk = ps_acc_pool.tile([P, HCH], F32, tag="po", bufs=2)
        nc.tensor.matmul(
            pk[:], wkt_a[:], xb0[:, sl],
            start=True, stop=False,
        )
        nc.tensor.matmul(
            pk[:], wkt_b[:], xb1[:, sl],
            start=False, stop=True,
        )
        nc.vector.tensor_scalar_add(k_rep[:, sl], pk[:], bk_t[:])

    # vT per j-tile: (n on partitions, channel on free) = x^T @ Wv^T + bv
    vt = big.tile([P, NJT * C], BF16, tag="vt")
    for jt in range(NJT):
        nsl = slice(jt * P, (jt + 1) * P)
        pv = ps_acc_pool.tile([P, C], F32, tag="po", bufs=2)
        nc.tensor.matmul(
            pv[:], xb0[:, nsl], wvt_a[:],
            start=True, stop=False,
        )
        nc.tensor.matmul(
            pv[:], xb1[:, nsl], wvt_b[:],
            start=False, stop=True,
        )
        nc.vector.tensor_tensor(
            vt[:, jt * C : (jt + 1) * C], pv[:], bv_t[:], op=ALU.add
        )

    # ---- attention main loop ---------------------------------------------
    for ich in range(NICH):
        i0 = ich * ICH
        # stage 1: s'[j,i] per j-tile, exp -> e tiles (bf16)
        etiles = []
        for jt in range(NJT):
            ksl = k_rep[0:DK, jt * P : (jt + 1) * P]
            ps = ps_s_pool.tile([P, ICH], F32, tag="ps_s")
            for h in range(ICH // HCH):
                qsl = q_rep[0:DK, i0 + h * HCH : i0 + (h + 1) * HCH]
                nc.tensor.matmul(
                    ps[:, h * HCH : (h + 1) * HCH], ksl, qsl,
                    start=True, stop=True,
                )
            e_t = epool.tile([P, ICH], BF16, tag="e")
            nc.scalar.activation(e_t[:], ps[:], AF.Exp)
            etiles.append(e_t)

        # stage 2a: out_un accumulation, full-width N=1024 streams
        po0 = ps_acc_pool.tile([P, ICH], F32, tag="po", bufs=2)
        po1 = ps_acc_pool.tile([P, ICH], F32, tag="po", bufs=2)
        for jt in range(NJT):
            st = jt == 0
            sp = jt == NJT - 1
            for h in range(ICH // HCH):
                hs = slice(h * HCH, (h + 1) * HCH)
                nc.tensor.matmul(
                    po0[:, hs], vt[:, jt * C : jt * C + P], etiles[jt][:, hs],
                    start=st, stop=sp,
                )
                nc.tensor.matmul(
                    po1[:, hs], vt[:, jt * C + P : (jt + 1) * C],
                    etiles[jt][:, hs], start=st, stop=sp,
                )

        # stage 2b: denominator, 4x column-tiled (quadrant q <- jt % 4)
        pd = ps_acc_pool.tile([P, ICH], F32, tag="pd", bufs=1)
        for jt in range(NJT):
            g = jt % 4
            for h in range(ICH // HCH):
                nc.tensor.matmul(
                    pd[32 * g : 32 * (g + 1), h * HCH : (h + 1) * HCH],
                    ones[:, 0:32],
                    etiles[jt][:, h * HCH : (h + 1) * HCH],
                    start=(jt < 4), stop=(jt >= NJT - 4),
                    tile_position=(0, 32 * g),
                )

        # stage 2c: quadrant-sum (broadcasts full denom to all partitions),
        # then y = gamma * out_un / den + x
        d_sb = fin.tile([P, ICH], F32, tag="d_sb")
        nc.vector.tensor_copy(d_sb[:], pd[:])
        for h in range(ICH // HCH):
            nc.tensor.matmul(
                pd[:, h * HCH : (h + 1) * HCH], ones_f[:],
                d_sb[:, h * HCH : (h + 1) * HCH], start=True, stop=True,
            )
        dr = fin.tile([P, ICH], F32, tag="dr")
        nc.vector.reciprocal(dr[:], pd[:])
        nc.vector.tensor_scalar_mul(dr[:], dr[:], gam_t[:])
        isl = slice(i0, i0 + ICH)
        t0 = fin.tile([P, ICH], F32, tag="t0")
        nc.vector.tensor_tensor(t0[:], po0[:], dr[:], op=ALU.mult)
        nc.vector.tensor_tensor(t0[:], t0[:], xf0[:, isl], op=ALU.add)
        nc.sync.dma_start(out=y_e[0:P, isl], in_=t0[:])
        t1 = fin.tile([P, ICH], F32, tag="t1")
        nc.vector.tensor_tensor(t1[:], po1[:], dr[:], op=ALU.mult)
        nc.vector.tensor_tensor(t1[:], t1[:], xf1[:, isl], op=ALU.add)
        nc.sync.dma_start(out=y_e[P : 2 * P, isl], in_=t1[:])


def build_bass(loop_n: int | None = None) -> bass.Bass:
    """Build the kernel. loop_n wraps the body in a device-side For_i loop
    (with a tiny 'tick' sentinel output) for slope-based benchmarking."""
    _apply_tile_patch()
    nc = bass.Bass()

    x_e = nc.declare_dram_parameter("x", [C, N], F32, isOutput=False)
    wqt_e = nc.declare_dram_parameter("wqt", [C, P], BF16, isOutput=False)
    wkt_e = nc.declare_dram_parameter("wkt", [C, P], BF16, isOutput=False)
    wvt_e = nc.declare_dram_parameter("wvt", [C, C], BF16, isOutput=False)
    bq_e = nc.declare_dram_parameter("bq_r", [P, 1], F32, isOutput=False)
    bk_e = nc.declare_dram_parameter("bk_r", [P, 1], F32, isOutput=False)
    bv_e = nc.declare_dram_parameter("bv_b", [P, C], F32, isOutput=False)
    gam_e = nc.declare_dram_parameter("gam_b", [P, 1], F32, isOutput=False)
    y_e = nc.declare_dram_parameter("y", [C, N], F32, isOutput=True)
    tick_e = None
    if loop_n is not None:
        tick_e = nc.declare_dram_parameter("tick", [1, 8], F32, isOutput=True)

    ext = (x_e, wqt_e, wkt_e, wvt_e, bq_e, bk_e, bv_e, gam_e, y_e)

    with (
        TileContext(nc) as tc,
        tc.tile_pool(name="consts", bufs=1) as consts,
        tc.tile_pool(name="big", bufs=1) as big,
        tc.tile_pool(name="epool", bufs=36) as epool,
        tc.tile_pool(name="fin", bufs=2) as fin,
        tc.tile_pool(name="ps_s", bufs=1, space="PSUM") as ps_s_pool,
        tc.tile_pool(name="ps_acc", bufs=3, space="PSUM") as ps_acc_pool,
    ):
        pools = (consts, big, epool, fin, ps_s_pool, ps_acc_pool)
        if loop_n is None:
            _emit_body(nc, tc, pools, ext)
        else:
            with tc.For_i(0, loop_n, 1):
                _emit_body(nc, tc, pools, ext)
            t = fin.tile([1, 8], F32, tag="tick")
            nc.vector.memset(t[:], 1.0)
            nc.sync.dma_start(out=tick_e[:], in_=t[:])

    return nc


_NC_CACHE = None


def _get_nc() -> bass.Bass:
    global _NC_CACHE
    if _NC_CACHE is None:
        _NC_CACHE = build_bass()
    return _NC_CACHE


def prep_core_inputs(x, Wq, bq, Wk, bk, Wv, bv, gamma):
    x = np.asarray(x, np.float32).reshape(B, C, N)
    wqt = np.ascontiguousarray(np.tile(np.asarray(Wq, np.float32).T, (1, 4))).astype(ml_dtypes.bfloat16)
    wkt = np.ascontiguousarray(np.tile(np.asarray(Wk, np.float32).T, (1, 4))).astype(ml_dtypes.bfloat16)
    wvt = np.ascontiguousarray(np.asarray(Wv, np.float32).T).astype(ml_dtypes.bfloat16)
    bq_r = np.ascontiguousarray(np.tile(np.asarray(bq, np.float32), 4)).reshape(P, 1)
    bk_r = np.ascontiguousarray(np.tile(np.asarray(bk, np.float32), 4)).reshape(P, 1)
    bv_b = np.ascontiguousarray(np.broadcast_to(np.asarray(bv, np.float32), (P, C)))
    # NOTE: quadrant strip-sum replicates each quadrant sum over 32 rows, so
    # the all-ones reduction yields 32x the true denominator; compensate here.
    gam_b = np.full((P, 1), 32.0 * float(np.asarray(gamma).reshape(-1)[0]), np.float32)
    shared = {
        "wqt": wqt, "wkt": wkt, "wvt": wvt,
        "bq_r": bq_r, "bk_r": bk_r, "bv_b": bv_b, "gam_b": gam_b,
    }
    return [{"x": np.ascontiguousarray(x[b]), **shared} for b in range(B)]


def kernel(**inputs) -> np.ndarray:
    nc = _get_nc()
    in_maps = prep_core_inputs(**inputs)
    res = run_bass_kernel_spmd(nc, in_maps, list(range(B)))
    y = np.stack([res.results[i]["y"] for i in range(B)])
    return np.ascontiguousarray(y.reshape(B, C, H, W).astype(np.float32))


# revision 17
# speedup vs baseline: 17.4550x; 1.2813x over previous
"""Trainium2 Bass kernel for nn_Attention_9594956939856.

Single-head spatial self-attention over 64x64 feature maps:
    q = Wq@x, k = Wk@x, v = Wv@x  (1x1 convs over channels)
    out = gamma * softmax(q^T k) @ v + x

Sharding: data-parallel over batch — 8 samples onto 8 NeuronCores, each core
computes one full sample (C=256, N=4096 tokens, dk=32). No collectives.

Per-core layout strategy (matmuls on TensorE compute out = lhsT.T @ rhs):
  - scores are computed directly TRANSPOSED: s'[j,i] = sum_d k[d,j] q[d,i]
    with k j-tiles stationary, so the huge attention matrix never needs a
    transpose. q/k are replicated 4x along partitions (via host-replicated
    W^T) so the K=32 contraction can later use 4x row-tiled matmuls.
  - softmax denominator: ones(128,128) stationary sums exp(s') over
    partitions (j), accumulated across j-tiles in PSUM; M=128 broadcasts the
    sum to every output partition for free.
  - v is produced directly in transposed layout vT[n,e] by the projection
    (lhsT = x chunks, rhs = Wv^T) — exactly the stationary layout the
    attention-weighted sum needs.
  - exp on ScalarE in (128,1024) chunks (bf16 out), fp32 PSUM accumulation.
    Scores are in [-5,5] for this input distribution, so softmax without
    max-subtraction is numerically safe.
"""

import ml_dtypes
import numpy as np

import concourse.bass as bass
import concourse.mybir as mybir
from concourse.tile import TileContext
from concourse.bass_utils import run_bass_kernel_spmd

B, C, H, W = 8, 256, 64, 64
N = H * W          # 4096 tokens
DK = C // 8        # 32
P = 128
F32 = mybir.dt.float32
F32R = mybir.dt.float32r  # fp32 storage, single-pass (4x faster) PE streaming
BF16 = mybir.dt.bfloat16
AF = mybir.ActivationFunctionType
ALU = mybir.AluOpType

NJT = N // P       # 32 j-tiles
ICH = 1024         # i-chunk width for the scores'/exp stage
NICH = N // ICH    # 4
HCH = 512          # accumulation sub-chunk (one PSUM bank)


# ---------------------------------------------------------------------------
# Workaround: the walrus build in this container allows only ONE sync wait
# per instruction ("Too many sync wait commands"), but Tile's wait
# assignment attaches up to 2 (and the tail drain more). Hoist all-but-one
# wait of any over-subscribed instruction onto dedicated same-engine nofuse
# nops inserted immediately before it in the ordered stream.
_PATCHED = False


def _apply_tile_patch():
    global _PATCHED
    if _PATCHED:
        return
    from concourse.tile import TileContext as TC
    from concourse.vector_clock import ScopedClock, VectorClock

    def _drain_and_barrier_split(self, tick_clock, wait_clock):
        gc = tick_clock.global_clock
        n = len(gc)
        for i in range(n):
            if gc[i] > 0:
                vec = [0] * n
                vec[i] = gc[i]
                ins = self.nc.sync.nop(nofuse=True, hint="tail_drain_wait")
                wait_clock.add_sem_waits(
                    ins.ins, ScopedClock({None: VectorClock(vec)})
                )
        self.nc.sync.drain()
        self.nc.all_engine_barrier()
        assert self.sems is not None
        popped = self.nc._tile_sem_poison_stack.pop()
        assert popped is self._sem_poison
        self.nc.clear_and_free_semaphores(list(self.sems.allocated().values()))
        self.nc.all_engine_barrier()

    TC._drain_and_barrier = _drain_and_barrier_split

    orig_lower = TC._lower_ordered_insts
    counter = [0]

    def _lower_split_waits(self, ordered):
        for bb_name, insts in ordered.items():
            new = []
            changed = False
            for inst in insts:
                si = inst.sync_info
                if si is not None and len(si.on_wait) > 1:
                    changed = True
                    waits = list(si.on_wait)
                    for w in waits[:-1]:
                        counter[0] += 1
                        new.append(
                            mybir.InstNoOp(
                                name=f"splitw-{counter[0]}",
                                sync_info=mybir.SyncInfo(
                                    on_wait=[w], on_update=[]
                                ),
                                bass_nofuse=True,
                                engine=inst.engine,
                            )
                        )
                    inst.sync_info = mybir.SyncInfo(
                        on_wait=[waits[-1]], on_update=list(si.on_update)
                    )
                new.append(inst)
            if changed:
                insts[:] = new
        return orig_lower(self, ordered)

    TC._lower_ordered_insts = _lower_split_waits
    _PATCHED = True


def _emit_body(nc, tc, pools, ext):
    """Emit one full attention computation (one sample)."""
    consts, big, epool, fin, ps_s_pool, ps_acc_pool = pools
    x_e, wqt_e, wkt_e, wvt_e, bq_e, bk_e, bv_e, gam_e, y_e = ext

    # ---- constants / weights ---------------------------------------------
    wqt_a = consts.tile([P, P], BF16, tag="wqt_a")
    wqt_b = consts.tile([P, P], BF16, tag="wqt_b")
    wkt_a = consts.tile([P, P], BF16, tag="wkt_a")
    wkt_b = consts.tile([P, P], BF16, tag="wkt_b")
    wvt_a = consts.tile([P, C], BF16, tag="wvt_a")
    wvt_b = consts.tile([P, C], BF16, tag="wvt_b")
    bq_t = consts.tile([P, 1], F32, tag="bq_t")
    bk_t = consts.tile([P, 1], F32, tag="bk_t")
    bv_t = consts.tile([P, C], F32, tag="bv_t")
    gam_t = consts.tile([P, 1], F32, tag="gam_t")
    ones = consts.tile([P, P], BF16, tag="ones")
    ones_f = consts.tile([P, P], F32, tag="ones_f")

    nc.sync.dma_start(out=wqt_a[:], in_=wqt_e[0:P, :])
    nc.sync.dma_start(out=wqt_b[:], in_=wqt_e[P : 2 * P, :])
    nc.sync.dma_start(out=wkt_a[:], in_=wkt_e[0:P, :])
    nc.sync.dma_start(out=wkt_b[:], in_=wkt_e[P : 2 * P, :])
    nc.sync.dma_start(out=wvt_a[:], in_=wvt_e[0:P, :])
    nc.sync.dma_start(out=wvt_b[:], in_=wvt_e[P : 2 * P, :])
    nc.sync.dma_start(out=bq_t[:], in_=bq_e[:])
    nc.sync.dma_start(out=bk_t[:], in_=bk_e[:])
    nc.sync.dma_start(out=bv_t[:], in_=bv_e[:])
    nc.sync.dma_start(out=gam_t[:], in_=gam_e[:])
    nc.vector.memset(ones[:], 1.0)
    nc.vector.memset(ones_f[:], 1.0)

    xf0 = big.tile([P, N], F32, tag="xf0")
    xf1 = big.tile([P, N], F32, tag="xf1")
    xb0 = big.tile([P, N], BF16, tag="xb0")
    xb1 = big.tile([P, N], BF16, tag="xb1")
    q_rep = big.tile([P, N], BF16, tag="q_rep")
    k_rep = big.tile([P, N], BF16, tag="k_rep")
    vt = big.tile([P, NJT * C], BF16, tag="vt")

    # ---- chunked x load + bf16 cast + projections (pipelined) ------------
    for nch in range(N // HCH):
        sl = slice(nch * HCH, (nch + 1) * HCH)
        nc.sync.dma_start(out=xf0[:, sl], in_=x_e[0:P, sl])
        nc.sync.dma_start(out=xf1[:, sl], in_=x_e[P : 2 * P, sl])
        nc.vector.tensor_copy(xb0[:, sl], xf0[:, sl])
        nc.vector.tensor_copy(xb1[:, sl], xf1[:, sl])
        pq = ps_acc_pool.tile([P, HCH], F32, tag="po", bufs=2)
        nc.tensor.matmul(pq[:], wqt_a[:], xb0[:, sl], start=True, stop=False)
        nc.tensor.matmul(pq[:], wqt_b[:], xb1[:, sl], start=False, stop=True)
        nc.vector.tensor_scalar_add(q_rep[:, sl], pq[:], bq_t[:])
        pk = ps_acc_pool.tile([P, HCH], F32, tag="po", bufs=2)
        nc.tensor.matmul(pk[:], wkt_a[:], xb0[:, sl], start=True, stop=False)
        nc.tensor.matmul(pk[:], wkt_b[:], xb1[:, sl], start=False, stop=True)
        nc.vector.tensor_scalar_add(k_rep[:, sl], pk[:], bk_t[:])
        # vT for the 4 j-tiles inside this chunk
        for jt in range(nch * 4, nch * 4 + 4):
            nsl = slice(jt * P, (jt + 1) * P)
            pv = ps_acc_pool.tile([P, C], F32, tag="po", bufs=2)
            nc.tensor.matmul(pv[:], xb0[:, nsl], wvt_a[:], start=True, stop=False)
            nc.tensor.matmul(pv[:], xb1[:, nsl], wvt_b[:], start=False, stop=True)
            nc.vector.tensor_tensor(
                vt[:, jt * C : (jt + 1) * C], pv[:], bv_t[:], op=ALU.add
            )

    # ---- attention main loop: i-chunks of 512 ----------------------------
    NCH = N // HCH  # 8
    for ich in range(NCH):
        isl = slice(ich * HCH, (ich + 1) * HCH)
        # stage 1: s' for 4 j-tiles at a time via 4x row-tiled matmuls into
        # one (128, 2048) PSUM tile, then a single exp over all 2048 cols.
        ebigs = []
        for jg in range(NJT // 4):
            ps_big = ps_s_pool.tile([P, 4 * HCH], F32, tag="ps_s")
            for g in range(4):
                jt = 4 * jg + g
                nc.tensor.matmul(
                    ps_big[:, g * HCH : (g + 1) * HCH],
                    k_rep[32 * g : 32 * (g + 1), jt * P : (jt + 1) * P],
                    q_rep[32 * g : 32 * (g + 1), isl],
                    start=True, stop=True,
                    tile_position=(32 * g, 0),
                )
            e_big = epool.tile([P, 4 * HCH], BF16, tag="e")
            nc.scalar.activation(e_big[:], ps_big[:], AF.Exp)
            ebigs.append(e_big)

        def esl(jt):
            return ebigs[jt // 4][:, (jt % 4) * HCH : (jt % 4 + 1) * HCH]

        # stage 2a: out_un accumulation (2 channel halves)
        po0 = ps_acc_pool.tile([P, HCH], F32, tag="po", bufs=2)
        po1 = ps_acc_pool.tile([P, HCH], F32, tag="po", bufs=2)
        for jt in range(NJT):
            st = jt == 0
            sp = jt == NJT - 1
            nc.tensor.matmul(
                po0[:], vt[:, jt * C : jt * C + P], esl(jt), start=st, stop=sp
            )
            nc.tensor.matmul(
                po1[:], vt[:, jt * C + P : (jt + 1) * C], esl(jt),
                start=st, stop=sp,
            )

        # stage 2b: denominator, 4x column-tiled (quadrant g = jt % 4)
        pd = ps_acc_pool.tile([P, HCH], F32, tag="pd", bufs=2)
        for jt in range(NJT):
            g = jt % 4
            nc.tensor.matmul(
                pd[32 * g : 32 * (g + 1), :], ones[:, 0:32], esl(jt),
                start=(jt < 4), stop=(jt >= NJT - 4),
                tile_position=(0, 32 * g),
            )

        # stage 2c: quadrant-sum (32x overcount folded into gamma), finalize
        d_sb = fin.tile([P, HCH], F32, tag="d_sb")
        nc.vector.tensor_copy(d_sb[:], pd[:])
        nc.tensor.matmul(pd[:], ones_f[:], d_sb[:], start=True, stop=True)
        dr = fin.tile([P, HCH], F32, tag="dr")
        nc.vector.reciprocal(dr[:], pd[:])
        nc.vector.tensor_scalar_mul(dr[:], dr[:], gam_t[:])
        t0 = fin.tile([P, HCH], F32, tag="t0")
        nc.vector.tensor_tensor(t0[:], po0[:], dr[:], op=ALU.mult)
        nc.vector.tensor_tensor(t0[:], t0[:], xf0[:, isl], op=ALU.add)
        nc.sync.dma_start(out=y_e[0:P, isl], in_=t0[:])
        t1 = fin.tile([P, HCH], F32, tag="t1")
        nc.vector.tensor_tensor(t1[:], po1[:], dr[:], op=ALU.mult)
        nc.vector.tensor_tensor(t1[:], t1[:], xf1[:, isl], op=ALU.add)
        nc.sync.dma_start(out=y_e[P : 2 * P, isl], in_=t1[:])


def build_bass(loop_n: int | None = None) -> bass.Bass:
    """Build the kernel. loop_n wraps the body in a device-side For_i loop
    (with a tiny 'tick' sentinel output) for slope-based benchmarking."""
    _apply_tile_patch()
    nc = bass.Bass()

    x_e = nc.declare_dram_parameter("x", [C, N], F32, isOutput=False)
    wqt_e = nc.declare_dram_parameter("wqt", [C, P], BF16, isOutput=False)
    wkt_e = nc.declare_dram_parameter("wkt", [C, P], BF16, isOutput=False)
    wvt_e = nc.declare_dram_parameter("wvt", [C, C], BF16, isOutput=False)
    bq_e = nc.declare_dram_parameter("bq_r", [P, 1], F32, isOutput=False)
    bk_e = nc.declare_dram_parameter("bk_r", [P, 1], F32, isOutput=False)
    bv_e = nc.declare_dram_parameter("bv_b", [P, C], F32, isOutput=False)
    gam_e = nc.declare_dram_parameter("gam_b", [P, 1], F32, isOutput=False)
    y_e = nc.declare_dram_parameter("y", [C, N], F32, isOutput=True)
    tick_e = None
    if loop_n is not None:
        tick_e = nc.declare_dram_parameter("tick", [1, 8], F32, isOutput=True)

    ext = (x_e, wqt_e, wkt_e, wvt_e, bq_e, bk_e, bv_e, gam_e, y_e)

    with (
        TileContext(nc) as tc,
        tc.tile_pool(name="consts", bufs=1) as consts,
        tc.tile_pool(name="big", bufs=1) as big,
        tc.tile_pool(name="epool", bufs=12) as epool,
        tc.tile_pool(name="fin", bufs=2) as fin,
        tc.tile_pool(name="ps_s", bufs=1, space="PSUM") as ps_s_pool,
        tc.tile_pool(name="ps_acc", bufs=3, space="PSUM") as ps_acc_pool,
    ):
        pools = (consts, big, epool, fin, ps_s_pool, ps_acc_pool)
        if loop_n is None:
            _emit_body(nc, tc, pools, ext)
        else:
            with tc.For_i(0, loop_n, 1):
                _emit_body(nc, tc, pools, ext)
            t = fin.tile([1, 8], F32, tag="tick")
            nc.vector.memset(t[:], 1.0)
            nc.sync.dma_start(out=tick_e[:], in_=t[:])

    return nc


_NC_CACHE = None


def _get_nc() -> bass.Bass:
    global _NC_CACHE
    if _NC_CACHE is None:
        _NC_CACHE = build_bass()
    return _NC_CACHE


def prep_core_inputs(x, Wq, bq, Wk, bk, Wv, bv, gamma):
    x = np.asarray(x, np.float32).reshape(B, C, N)
    wqt = np.ascontiguousarray(np.tile(np.asarray(Wq, np.float32).T, (1, 4))).astype(ml_dtypes.bfloat16)
    wkt = np.ascontiguousarray(np.tile(np.asarray(Wk, np.float32).T, (1, 4))).astype(ml_dtypes.bfloat16)
    wvt = np.ascontiguousarray(np.asarray(Wv, np.float32).T).astype(ml_dtypes.bfloat16)
    bq_r = np.ascontiguousarray(np.tile(np.asarray(bq, np.float32), 4)).reshape(P, 1)
    bk_r = np.ascontiguousarray(np.tile(np.asarray(bk, np.float32), 4)).reshape(P, 1)
    bv_b = np.ascontiguousarray(np.broadcast_to(np.asarray(bv, np.float32), (P, C)))
    # NOTE: quadrant strip-sum replicates each quadrant sum over 32 rows, so
    # the all-ones reduction yields 32x the true denominator; compensate here.
    gam_b = np.full((P, 1), 32.0 * float(np.asarray(gamma).reshape(-1)[0]), np.float32)
    shared = {
        "wqt": wqt, "wkt": wkt, "wvt": wvt,
        "bq_r": bq_r, "bk_r": bk_r, "bv_b": bv_b, "gam_b": gam_b,
    }
    return [{"x": np.ascontiguousarray(x[b]), **shared} for b in range(B)]


def kernel(**inputs) -> np.ndarray:
    nc = _get_nc()
    in_maps = prep_core_inputs(**inputs)
    res = run_bass_kernel_spmd(nc, in_maps, list(range(B)))
    y = np.stack([res.results[i]["y"] for i in range(B)])
    return np.ascontiguousarray(y.reshape(B, C, H, W).astype(np.float32))
